# revision 1
# baseline (speedup 1.0000x reference)
"""Causal self-attention (B=4, T=2048, C=1024, H=16) on 8 trn2 NeuronCores.

Sharding: core c = (batch b = c // 2, head-group hg = c % 2). Each core runs
one batch with 8 of the 16 heads: column-parallel c_attn, full causal
attention for its heads, row-parallel c_proj producing a partial [T, C]
output. The host sums the two head-group partials per batch (the row-parallel
all-reduce is folded into the unshard step).

Per-core kernel (bass/Tile):
  - x is loaded fp32, cast to bf16, transposed 128x128-blockwise on the PE
    (via identity) into xT [c, t] layout.
  - QKV^T projection with lhsT=W (column-sliced), rhs=xT: gives Q^T/K^T in
    [d, t] layout directly - no per-block transposes anywhere in attention.
    V is projected separately in [t, j] layout (lhsT=xT blocks).
  - Attention computes S^T[k, q] blocks (lhsT=K^T, rhs=Q^T, contraction d=64),
    exp on the scalar engine with the 1/sqrt(d) folded into the activation
    scale (no max subtraction: scores are O(1) by construction), causal
    masking by a sliding precomputed bf16 mask multiply on the vector engine,
    then att@V with lhsT=[V_h | ones]: PSUM rows 0:64 accumulate the
    unnormalized output^T, rows 64:128 the softmax denominator, pre-broadcast.
  - Normalize with a fast-reciprocal + multiply (vector engine), c_proj from
    attT (lhsT) with row-sliced W_proj, bias adds fused into PSUM->SBUF copies.

All matmuls are bf16 with fp32 PSUM accumulation (scale-relative absmax vs
the fp32 reference ~3e-3).
"""

from contextlib import ExitStack

import ml_dtypes
import numpy as np

import concourse.bass as bass
import concourse.mybir as mybir
from concourse import bacc
from concourse.bass_utils import run_bass_kernel_spmd
from concourse.masks import make_identity
from concourse.tile import TileContext

F32 = mybir.dt.float32
BF16 = mybir.dt.bfloat16

P = 128
D = 64          # head dim
HG = 8          # heads per core
JQ = HG * D     # 512 j-channels per q/k/v section per core
C = 1024        # model dim
B = 4
T = 2048
KSUB = C // P   # 8
JT_Q = JQ // P  # 4
QTILE = 512
KGRP = 3        # k-blocks per exp group (3 PSUM banks)
N_CORES = 8


def _build_nc():
    nc = bacc.Bacc("TRN2", target_bir_lowering=False, debug=False)

    TT = T // P
    NQT = T // QTILE

    x = nc.dram_tensor("x", [T, C], F32, kind="ExternalInput")
    wqk = nc.dram_tensor("wqk", [C, 2 * JQ], BF16, kind="ExternalInput")
    wv = nc.dram_tensor("wv", [C, JQ], BF16, kind="ExternalInput")
    wp = nc.dram_tensor("wp", [JQ, C], BF16, kind="ExternalInput")
    battn = nc.dram_tensor("battn", [3 * JQ], F32, kind="ExternalInput")
    bproj = nc.dram_tensor("bproj", [C], F32, kind="ExternalInput")
    out = nc.dram_tensor("out", [T, C], F32, kind="ExternalOutput")

    with TileContext(nc) as tc, ExitStack() as ctx:
        consts = ctx.enter_context(tc.tile_pool(name="consts", bufs=1))
        wpool = ctx.enter_context(tc.tile_pool(name="wpool", bufs=1))
        big = ctx.enter_context(tc.tile_pool(name="big", bufs=1))

        ident = consts.tile([P, P], BF16)
        make_identity(nc, ident[:])

        # sliding causal mask M[p, g] = 1.0 iff g - p - 384 >= 0; diagonal
        # k-block at offset dj uses slice [384-128*dj : 896-128*dj]
        maskw = QTILE + 384
        cmask = consts.tile([P, maskw], BF16)
        nc.gpsimd.memset(cmask[:], 1.0)
        nc.gpsimd.affine_select(
            out=cmask[:],
            in_=cmask[:],
            compare_op=mybir.AluOpType.is_ge,
            fill=0.0,
            base=-384,
            pattern=[[1, maskw]],
            channel_multiplier=-1,
        )

        battn_sb = consts.tile([P, 3 * JQ // P], F32)
        nc.sync.dma_start(battn_sb[:], battn.rearrange("(a p) -> p a", p=P))
        bv_row = consts.tile([1, JQ], F32)
        nc.sync.dma_start(bv_row[:], battn[None, 2 * JQ:])
        bv_bc = consts.tile([P, JQ], F32)
        nc.gpsimd.partition_broadcast(bv_bc[:], bv_row[:])
        bp_row = consts.tile([1, C], F32)
        nc.sync.dma_start(bp_row[:], bproj[None, :])
        bp_bc = consts.tile([P, C], F32)
        nc.gpsimd.partition_broadcast(bp_bc[:], bp_row[:])

        wqk_sb = wpool.tile([P, KSUB, 2 * JQ], BF16)
        nc.sync.dma_start(wqk_sb[:], wqk.rearrange("(ko ki) j -> ki ko j", ki=P))
        wv_sb = wpool.tile([P, KSUB, JQ], BF16)
        nc.sync.dma_start(wv_sb[:], wv.rearrange("(ko ki) j -> ki ko j", ki=P))
        wp_sb = wpool.tile([P, JQ // P, C], BF16)
        nc.sync.dma_start(wp_sb[:], wp.rearrange("(ko ki) j -> ki ko j", ki=P))

        xT = big.tile([P, KSUB, T], BF16)
        QT = big.tile([P, JT_Q, T], BF16)
        KT = big.tile([P, JT_Q, T], BF16)
        V = big.tile([P, TT, HG, 2, D], BF16)
        nc.vector.memset(V[:, :, :, 0, :], 1.0)

        # ===== phase A: load x, cast bf16, PE-transpose into xT ========
        with tc.tile_pool(name="xstage", bufs=3) as xstage, tc.tile_pool(
            name="tpsum", bufs=4, space="PSUM"
        ) as tpsum:
            for tt in range(TT):
                xf = xstage.tile([P, C], F32, tag="xf")
                nc.sync.dma_start(xf[:], x[tt * P : (tt + 1) * P, :])
                xb = xstage.tile([P, C], BF16, tag="xb")
                nc.vector.tensor_copy(xb[:], xf[:])
                for ks in range(KSUB):
                    tp = tpsum.tile([P, P], BF16)
                    nc.tensor.transpose(tp[:], xb[:, ks * P : (ks + 1) * P], ident[:])
                    nc.vector.tensor_copy(xT[:, ks, tt * P : (tt + 1) * P], tp[:])

        # ===== phase B: projections ====================================
        with tc.tile_pool(name="ppsum", bufs=4, space="PSUM") as ppsum:
            for tt in range(TT):  # V in [t, j] layout
                ps = ppsum.tile([P, JQ], F32, tag="vps")
                for ks in range(KSUB):
                    nc.tensor.matmul(
                        ps[:],
                        lhsT=xT[:, ks, tt * P : (tt + 1) * P],
                        rhs=wv_sb[:, ks, :],
                        start=(ks == 0),
                        stop=(ks == KSUB - 1),
                    )
                nc.vector.tensor_add(V[:, tt, :, 1, :], ps[:], bv_bc[:])

            for jt in range(2 * JT_Q):  # Q^T / K^T in [j, t] layout
                dst = QT if jt < JT_Q else KT
                js = jt % JT_Q
                bias_col = battn_sb[:, jt : jt + 1]
                for tq in range(T // QTILE):
                    ps = ppsum.tile([P, QTILE], F32, tag="qkps")
                    for ks in range(KSUB):
                        nc.tensor.matmul(
                            ps[:],
                            lhsT=wqk_sb[:, ks, jt * P : (jt + 1) * P],
                            rhs=xT[:, ks, tq * QTILE : (tq + 1) * QTILE],
                            start=(ks == 0),
                            stop=(ks == KSUB - 1),
                        )
                    nc.vector.tensor_scalar_add(
                        dst[:, js, tq * QTILE : (tq + 1) * QTILE], ps[:], bias_col
                    )

        # ===== phase C: attention + c_proj per q-tile ==================
        with tc.tile_pool(name="stpsum", bufs=2, space="PSUM") as stpsum, \
             tc.tile_pool(name="avpsum", bufs=2, space="PSUM") as avpsum, \
             tc.tile_pool(name="et", bufs=3) as etpool, \
             tc.tile_pool(name="rcp", bufs=2) as rcpool, \
             tc.tile_pool(name="attw", bufs=2) as attw, \
             tc.tile_pool(name="ostage", bufs=3) as ostage:
            for qt in range(NQT):
                nkb = 4 * (qt + 1)
                attT = attw.tile([P, JT_Q, QTILE], BF16)
                for h in range(HG):
                    hrow = (h % 2) * D
                    js = h // 2
                    kt_l = KT[hrow : hrow + D, js, :]
                    qt_l = QT[hrow : hrow + D, js, qt * QTILE : (qt + 1) * QTILE]
                    av = avpsum.tile([P, QTILE], F32, tag="av")
                    ngrp = (nkb + KGRP - 1) // KGRP
                    for g in range(ngrp):
                        k0 = g * KGRP
                        kn = min(KGRP, nkb - k0)
                        st = stpsum.tile([P, KGRP, QTILE], F32, tag="st")
                        for j in range(kn):
                            kb = k0 + j
                            nc.tensor.matmul(
                                st[:, j, :],
                                lhsT=kt_l[:, kb * P : (kb + 1) * P],
                                rhs=qt_l,
                                start=True,
                                stop=True,
                            )
                        et = etpool.tile([P, KGRP, QTILE], BF16, tag="et")
                        nc.scalar.activation(
                            et[:, :kn, :],
                            st[:, :kn, :],
                            mybir.ActivationFunctionType.Exp,
                            scale=float(1.0 / np.sqrt(D)),
                        )
                        for j in range(kn):
                            kb = k0 + j
                            dj = kb - 4 * qt
                            if dj >= 0:
                                s = 384 - 128 * dj
                                nc.vector.tensor_mul(
                                    et[:, j, :], et[:, j, :], cmask[:, s : s + QTILE]
                                )
                            nc.tensor.matmul(
                                av[:],
                                lhsT=V[:, kb, h],
                                rhs=et[:, j, :],
                                start=(kb == 0),
                                stop=(kb == nkb - 1),
                            )
                    rc = rcpool.tile([D, QTILE], F32, tag="rc")
                    nc.vector.reciprocal_approx_fast(rc[:], av[:D, :])
                    nc.vector.tensor_mul(attT[hrow : hrow + D, js, :], av[D:, :], rc[:])

                for tl in range(QTILE // P):
                    tt = qt * (QTILE // P) + tl
                    ot = ostage.tile([P, C], F32, tag="ot")
                    for nt in range(C // QTILE):
                        ps = avpsum.tile([P, QTILE], F32, tag="av")
                        for js2 in range(JT_Q):
                            nc.tensor.matmul(
                                ps[:],
                                lhsT=attT[:, js2, tl * P : (tl + 1) * P],
                                rhs=wp_sb[:, js2, nt * QTILE : (nt + 1) * QTILE],
                                start=(js2 == 0),
                                stop=(js2 == JT_Q - 1),
                            )
                        nc.vector.tensor_add(
                            ot[:, nt * QTILE : (nt + 1) * QTILE],
                            ps[:],
                            bp_bc[:, nt * QTILE : (nt + 1) * QTILE],
                        )
                    nc.sync.dma_start(out[tt * P : (tt + 1) * P, :], ot[:])

    nc.compile()
    return nc


_NC_CACHE = {}


def _get_nc():
    if "nc" not in _NC_CACHE:
        _NC_CACHE["nc"] = _build_nc()
    return _NC_CACHE["nc"]


def _core_inputs(x, W_attn, b_attn, W_proj, b_proj, b, hg):
    bf = ml_dtypes.bfloat16
    qs = slice(hg * JQ, (hg + 1) * JQ)
    ks = slice(C + hg * JQ, C + (hg + 1) * JQ)
    vs = slice(2 * C + hg * JQ, 2 * C + (hg + 1) * JQ)
    return {
        "x": np.ascontiguousarray(x[b], dtype=np.float32),
        "wqk": np.ascontiguousarray(
            np.concatenate([W_attn[:, qs], W_attn[:, ks]], axis=1)
        ).astype(bf),
        "wv": np.ascontiguousarray(W_attn[:, vs]).astype(bf),
        "wp": np.ascontiguousarray(W_proj[hg * JQ : (hg + 1) * JQ, :]).astype(bf),
        "battn": np.ascontiguousarray(
            np.concatenate([b_attn[qs], b_attn[ks], b_attn[vs]])
        ).astype(np.float32),
        "bproj": np.asarray(b_proj, dtype=np.float32),
    }


def kernel(x, W_attn, b_attn, W_proj, b_proj):
    x = np.asarray(x, dtype=np.float32)
    W_attn = np.asarray(W_attn, dtype=np.float32)
    b_attn = np.asarray(b_attn, dtype=np.float32)
    W_proj = np.asarray(W_proj, dtype=np.float32)
    b_proj = np.asarray(b_proj, dtype=np.float32)

    nc = _get_nc()
    in_maps = [
        _core_inputs(x, W_attn, b_attn, W_proj, b_proj, b=c // 2, hg=c % 2)
        for c in range(N_CORES)
    ]
    res = run_bass_kernel_spmd(nc, in_maps, core_ids=list(range(N_CORES)))
    out = np.empty((B, T, C), dtype=np.float32)
    for b in range(B):
        out[b] = res.results[2 * b]["out"] + res.results[2 * b + 1]["out"]
    return out

